# revision 63
# baseline (speedup 1.0000x reference)
"""Trainium2 Bass kernel for nn_AttentionWithVQ (B=4, N=2048, DIM=512, H=8,
depthwise-conv positional term, softmax attention, output projection).

Sharding: data-parallel over B (4 batches x 2 core-groups) and tensor-parallel
over heads (4 heads per core) -> 8 cores, fully independent per core except a
final partial-sum over the two head-groups of each batch, done on host at
gather time (the output projection contracts over heads).

Core algorithmic fusion: the score matrix
    S = 0.5*(scale * q @ k^T + scale * conv1(m) @ conv2(s)^T)
is ONE matmul over a concatenated 128-feature axis:
    S = Qp^T @ Kp,  Qp = [q*scale*0.5 ; conv1(m)*scale*0.5], Kp = [k ; conv2(s)]
which exactly fills the 128x128 PE array contraction dim.

Softmax denominators come for free by appending a ones-column to V
(attn@[V|1] yields the row-sums of exp(S) in the last output row); exp() is
numerically safe without max-subtraction for this problem's score magnitudes
(|S| < ~1 given the 0.02-scaled weights).

Partition alignment: compute engines are lane-locked (PSUM partition p ->
SBUF partition p), so per-head feature layouts alternate by head parity
(even heads [qk;conv], odd heads [conv;qk]) making every PSUM->SBUF copy
partition-aligned; the few genuinely shifting copies (odd-head attention
outputs, denominator rows) go through DMA, which can move partitions freely.
All permutation bookkeeping is done host-side in numpy when preparing
per-core inputs.

Schedule (v2): the kernel is a single software-pipelined stream ordered to
keep the PE and ACT engines saturated end-to-end:
  - input DMAs are split across the SP/ACT/Pool queues with the
    qkv-critical tensors (xt, wqk) first;
  - qkv bias-adds run on the ACT engine (idle until the first exp);
  - attention runs stripe-outer (q 1024-blocks) / head-inner, with
    scores(nk+1) emitted before attnV(nk) so exp latency is hidden, the
    v-projection matmuls interleaved into the first head's window and the
    second half of the q/k projection into the second head's window;
  - each stripe is normalized, projected, and DMA'd out as soon as its
    4 heads finish, overlapping the next stripe's attention.
"""

import os
import sys

sys.path.insert(0, "/opt/trn_rl_repo")

import numpy as np

# ---------------------------------------------------------------- constants
B, N, DIM, HEAD, VQE_K = 4, 2048, 512, 8, 3
Dh = DIM // HEAD            # 64
HPC = HEAD // 2             # heads per core (8 cores = 4 batch * 2 groups)
P = 128
FB = 512                    # one fp32 PSUM bank
NKB = N // P                # 16
SCALE_Q = Dh ** -0.5 * 0.5  # folds the 0.5 score scale into the q/conv1 side

_DEFAULT_CFG = {"qkv": "bf16", "attn": "bf16", "proj": "bf16", "av": "same"}

_CACHE = {}


def _np_dt(v):
    if v == "bf16":
        import ml_dtypes
        return ml_dtypes.bfloat16
    return np.float32


# ---------------------------------------------------------------- host prep
def _host_prep(core, inp, cfg=None):
    """Build the per-core input arrays (sharding + layout permutations)."""
    cfg = cfg or _DEFAULT_CFG
    b, g = core // 2, core % 2
    f32 = np.float32
    x, m, s = inp["x"], inp["m"], inp["s"]
    qkv_w, qkv_b = inp["qkv_w"], inp["qkv_b"]
    proj_w = inp["proj_w"]
    p1w = inp["pe1_w"].reshape(HEAD, VQE_K)
    p2w = inp["pe2_w"].reshape(HEAD, VQE_K)
    pe1_b, pe2_b = inp["pe1_b"], inp["pe2_b"]
    dt_qkv = _np_dt(cfg["qkv"])
    dt_proj = _np_dt(cfg["proj"])
    dt_conv = _np_dt("bf16" if cfg["attn"] == "bf16" else "f32")

    def fold(a):
        """[k*128, C] -> [128, k*C]: partition p holds rows p, 128+p, ...
        concatenated, so each partition's DMA data is one contiguous run."""
        k = a.shape[0] // 128
        return np.ascontiguousarray(
            np.concatenate([a[c * 128:(c + 1) * 128] for c in range(k)],
                           axis=1))

    d = {}
    d["xt"] = fold(x[b].T.astype(dt_qkv))  # [128, 4*2048]

    # m/s transposed, tile t rows = [head(2t+1) feats ; head(2t) feats]
    mt = np.empty((256, N), f32)
    st = np.empty((256, N), f32)
    mcw = np.zeros((128, 8), f32)
    scw = np.zeros((128, 8), f32)
    for t in range(2):
        h_lo, h_hi = g * 4 + 2 * t + 1, g * 4 + 2 * t
        mt[t * 128:t * 128 + 64] = m[b][:, h_lo * 64:(h_lo + 1) * 64].T
        mt[t * 128 + 64:t * 128 + 128] = m[b][:, h_hi * 64:(h_hi + 1) * 64].T
        st[t * 128:t * 128 + 64] = s[b][:, h_lo * 64:(h_lo + 1) * 64].T
        st[t * 128 + 64:t * 128 + 128] = s[b][:, h_hi * 64:(h_hi + 1) * 64].T
        for p in range(128):
            h = g * 4 + 2 * t + (1 if p < 64 else 0)
            mcw[p, 4 * t:4 * t + 3] = p1w[h] * SCALE_Q
            scw[p, 4 * t:4 * t + 3] = p2w[h]
            mcw[p, 4 * t + 3] = pe1_b[h] * SCALE_Q
            scw[p, 4 * t + 3] = pe2_b[h]
    d["mt"], d["st"] = fold(mt.astype(dt_conv)), fold(st.astype(dt_conv))
    d["mcw"], d["scw"] = mcw, scw

    # q/k projection weights: chunk ch=(t, q|k) = [even-head rows; odd-head rows]
    wqk_f = np.empty((512, DIM), f32)
    qkb = np.zeros((128, 4), f32)
    for t in range(2):
        for j in range(2):  # 0=q, 1=k
            ch = 2 * t + j
            h_e, h_o = g * 4 + 2 * t, g * 4 + 2 * t + 1
            base = j * DIM
            wqk_f[ch * 128:ch * 128 + 64] = qkv_w[base + h_e * 64:base + (h_e + 1) * 64]
            wqk_f[ch * 128 + 64:(ch + 1) * 128] = qkv_w[base + h_o * 64:base + (h_o + 1) * 64]
            qkb[0:64, ch] = qkv_b[base + h_e * 64:base + (h_e + 1) * 64]
            qkb[64:128, ch] = qkv_b[base + h_o * 64:base + (h_o + 1) * 64]
            if j == 0:
                wqk_f[ch * 128:(ch + 1) * 128] *= SCALE_Q
                qkb[:, ch] *= SCALE_Q
    d["wqk"] = fold(wqk_f.T.astype(dt_qkv))  # [128, 4*512]
    d["qkb"] = qkb

    d["wv"] = fold(np.ascontiguousarray(
        qkv_w[2 * DIM + g * 256:2 * DIM + (g + 1) * 256].T).astype(dt_qkv))

    # proj rows / v-bias in aT partition order:
    # aT tile t partition p -> head 2t+(p>=64), d=p%64
    pjt = np.empty((256, DIM), f32)
    vbv = np.empty((256,), f32)
    for t in range(2):
        for p in range(128):
            h_l = 2 * t + (1 if p >= 64 else 0)
            h = g * 4 + h_l
            pjt[t * 128 + p] = proj_w[:, h * 64 + (p % 64)]
            vbv[t * 128 + p] = qkv_b[2 * DIM + h * 64 + (p % 64)]
    d["pjt"] = fold(pjt.astype(dt_proj))  # [128, 2*512]
    d["vbv"] = np.ascontiguousarray(vbv.reshape(2, 128).T.astype(f32))  # [128, 2]
    return d


# ------------------------------------------------------------- device build
def _emit(tc, nc, io, cfg):
    from contextlib import ExitStack

    from concourse import mybir

    dt = mybir.dt
    f32 = dt.float32
    AF = mybir.ActivationFunctionType
    ALU = mybir.AluOpType

    def _dt(v):
        return {"f32": f32, "f32r": dt.float32r, "bf16": dt.bfloat16}[v]

    dt_qkv = _dt(cfg["qkv"])
    dt_attn = _dt(cfg["attn"])
    dt_proj = _dt(cfg["proj"])
    dt_conv = dt.bfloat16 if dt_attn == dt.bfloat16 else f32
    # av=f8: exp writes fp8e4m3 and attnV runs DoubleRow (K=256/instr,
    # 2x PE throughput); scores/QP/KP stay in dt_attn
    AV8 = cfg.get("av") == "f8"
    dt_av = dt.float8e4 if AV8 else dt_attn
    VW = 68 if AV8 else 66  # per-head V block width (4B-aligned offsets)
    # exp granularity: one ACT op per FBS-wide stripe (matmuls within are
    # still 512-wide: a matmul output cannot cross a PSUM bank)
    FBS = 1024
    NIH = FBS // FB           # 2
    NQ2 = N // FBS            # 2 q-stripes

    with ExitStack() as ctx:
        persist = ctx.enter_context(tc.tile_pool(name="persist", bufs=1))

        # ---- persistent tiles
        QP, KP, v_sb, aT = [], [], [], []
        # critical-path inputs first: xt on the SP queue, wqk on the ACT
        # queue so both stream concurrently from t=0
        # inputs are host-folded to [128, k*cols] (each partition's bytes
        # one contiguous DRAM run -> ~4x fewer DMA descriptors); weight
        # tiles first on each HWDGE queue, xt halves split across both
        xt_big = persist.tile([128, 4 * N], dt_qkv, name="xt", tag="xt")
        xt_sb = [xt_big[:, c * N:(c + 1) * N] for c in range(4)]
        wqk_big = persist.tile([128, 4 * 512], dt_qkv, name="wqk", tag="wqk")
        wqk_sb = [wqk_big[:, c * 512:(c + 1) * 512] for c in range(4)]
        wv_big = persist.tile([128, 4 * 256], dt_qkv, name="wv", tag="wv")
        wv_sb = [wv_big[:, c * 256:(c + 1) * 256] for c in range(4)]
        # pieces sized/ordered to match the qkv c-loop consumption order
        nc.scalar.dma_start(wqk_big[:, 0:1024], io["wqk"][:, 0:1024])
        nc.sync.dma_start(xt_big[:, 0:N], io["xt"][:, 0:N])
        nc.scalar.dma_start(wqk_big[:, 1024:2048], io["wqk"][:, 1024:2048])
        nc.sync.dma_start(xt_big[:, N:2 * N], io["xt"][:, N:2 * N])
        nc.scalar.dma_start(xt_big[:, 2 * N:3 * N], io["xt"][:, 2 * N:3 * N])
        nc.gpsimd.dma_start(wv_big[:], io["wv"][:, :])
        nc.scalar.dma_start(xt_big[:, 3 * N:4 * N], io["xt"][:, 3 * N:4 * N])
        mcw_sb = persist.tile([128, 8], f32, name="mcw", tag="mcw")
        nc.gpsimd.dma_start(mcw_sb[:], io["mcw"][:, :])
        scw_sb = persist.tile([128, 8], f32, name="scw", tag="scw")
        nc.gpsimd.dma_start(scw_sb[:], io["scw"][:, :])
        qkb_sb = persist.tile([128, 4], f32, name="qkb", tag="qkb")
        nc.sync.dma_start(qkb_sb[:], io["qkb"][:, :])

        for h in range(HPC):
            QP.append(persist.tile([128, N], dt_attn, name=f"QP{h}", tag=f"QP{h}"))
            KP.append(persist.tile([128, N], dt_attn, name=f"KP{h}", tag=f"KP{h}"))
        # per-head V block is [v(64) | ones | zero-pad] = VW columns: matmul
        # operands need 4-byte-aligned offsets, so the block width must keep
        # h*VW*dtsize 4-aligned (66 for 2-byte dtypes, 68 for fp8)
        if AV8:
            for p in range(NKB // 2):
                v_sb.append(persist.tile([128, 2 * HPC * VW], dt_av,
                                         name=f"vsb{p}", tag=f"vsb{p}"))
        else:
            for blk in range(NKB):
                v_sb.append(persist.tile([128, HPC * VW], dt_attn,
                                         name=f"vsb{blk}", tag=f"vsb{blk}"))
        for t in range(2):
            aT.append(persist.tile([128, N], dt_proj, name=f"aT{t}", tag=f"aT{t}"))
        # softmax denominators, DMA-reshaped onto 16-partition tiles so the
        # (slow per-element) reciprocal runs 16x wider than a row layout;
        # head h at rows h*32 (32-aligned for the DVE), stripe in columns
        denR = persist.tile([128, 128], f32, name="denR", tag="denR")
        # stationary 0/1 patterns for the denominator-broadcast matmul:
        # lhsT = bcpat[64:65, par*128:(par+1)*128] replicates a [1, FB] den
        # row onto the head's 64-partition half of a PSUM tile (bf16: the
        # BIR verifier requires f32r operands to be produced as f32r, which
        # memset/copy cannot do)
        bcpat = persist.tile([128, 256], dt.bfloat16, name="bcpat",
                             tag="bcpat")
        nc.vector.memset(bcpat[:, 0:64], 1.0)
        nc.vector.memset(bcpat[:, 64:192], 0.0)
        nc.vector.memset(bcpat[:, 192:256], 1.0)

        vbv_sb = persist.tile([128, 2], f32, name="vbv", tag="vbv")
        nc.sync.dma_start(vbv_sb[:], io["vbv"][:, :])
        pjt_big = persist.tile([128, 2 * 512], dt_proj, name="pjt", tag="pjt")
        pjt_sb = [pjt_big[:, f * 512:(f + 1) * 512] for f in range(2)]
        nc.scalar.dma_start(pjt_big[:], io["pjt"][:, :])

        # ---- depthwise convs (DVE, bf16 2x mode; inputs on the Pool queue)
        # PE p-state warm-up: ~10 back-to-back dummy matmuls on the (tiny,
        # already-memset) bcpat tile during the input-DMA wait, so the real
        # qkv matmuls start at full clock instead of ramping through them
        with tc.tile_pool(name="ps_warm", bufs=1, space="PSUM") as ps_wp:
            wps = ps_wp.tile([128, 256], f32, name="warm", tag="warm")
            for _ in range(10):
                nc.tensor.matmul(wps[:], bcpat[:, 0:128], bcpat[:, 0:256],
                                 start=True, stop=True)


        # t=0 convs first: heads 0/1 gate the first attention window; the
        # folded [128, 2N] inputs are loaded whole and convolved in place
        mt_sb = persist.tile([128, 2 * N], dt_conv, name="mt", tag="mt")
        nc.gpsimd.dma_start(mt_sb[:], io["mt"][:, :])
        st_sb = persist.tile([128, 2 * N], dt_conv, name="st", tag="st")
        nc.gpsimd.dma_start(st_sb[:], io["st"][:, :])
        with tc.tile_pool(name="conv", bufs=2) as convp:
            for t in range(2):
                for src_sb, wv_, dst in ((mt_sb, mcw_sb, QP),
                                         (st_sb, scw_sb, KP)):
                    xin = src_sb[:, t * N:(t + 1) * N]
                    y = convp.tile([128, N], dt_conv, name=f"cy_{t}",
                                   tag="cy", bufs=2)
                    w0, w1, w2, cb = (wv_[:, 4 * t + k:4 * t + k + 1]
                                      for k in range(4))
                    nc.vector.tensor_scalar(y[:], xin[:], w1, cb,
                                            ALU.mult, ALU.add)
                    nc.vector.scalar_tensor_tensor(
                        y[:, 1:], xin[:, :N - 1], w0, y[:, 1:],
                        ALU.mult, ALU.add)
                    nc.vector.scalar_tensor_tensor(
                        y[:, :N - 1], xin[:, 1:], w2, y[:, :N - 1],
                        ALU.mult, ALU.add)
                    nc.vector.tensor_copy(dst[2 * t + 1][0:64, :], y[0:64, :])
                    nc.vector.tensor_copy(dst[2 * t][64:128, :], y[64:128, :])

        # ---- q/k projections, first half (chunks 0,1 = heads 0,1).
        # Bias-adds run on ACT (idle until the first exp); the PSUM pool
        # closes before attention so its banks are reusable there.
        def qk_chunk_matmuls(ps, ch, qb):
            for ih in range(NIH):
                hqs = slice(qb * FBS + ih * FB, qb * FBS + (ih + 1) * FB)
                for c in range(4):
                    nc.tensor.matmul(
                        ps[:, ih * FB:(ih + 1) * FB],
                        wqk_sb[c][:, ch * 128:(ch + 1) * 128],
                        xt_sb[c][:, hqs],
                        start=(c == 0), stop=(c == 3))

        def qk_bias_out(ps, ch, qb, engine):
            t = ch // 2
            qs = slice(qb * FBS, (qb + 1) * FBS)
            dst = QP if ch % 2 == 0 else KP
            if engine == "act":
                nc.scalar.activation(dst[2 * t][0:64, qs], ps[0:64, :],
                                     AF.Identity, bias=qkb_sb[0:64, ch:ch + 1])
                nc.scalar.activation(dst[2 * t + 1][64:128, qs], ps[64:128, :],
                                     AF.Identity,
                                     bias=qkb_sb[64:128, ch:ch + 1])
            elif engine == "pool":
                nc.gpsimd.tensor_scalar_add(dst[2 * t][0:64, qs],
                                            ps[0:64, :],
                                            qkb_sb[0:64, ch:ch + 1])
                nc.gpsimd.tensor_scalar_add(dst[2 * t + 1][64:128, qs],
                                            ps[64:128, :],
                                            qkb_sb[64:128, ch:ch + 1])
            else:
                nc.vector.tensor_scalar_add(dst[2 * t][0:64, qs], ps[0:64, :],
                                            qkb_sb[0:64, ch:ch + 1])
                nc.vector.tensor_scalar_add(dst[2 * t + 1][64:128, qs],
                                            ps[64:128, :],
                                            qkb_sb[64:128, ch:ch + 1])

        # c-OUTER accumulation over 4 live PSUM tiles (8 banks): matmuls for
        # xt block c start as soon as that block's DMA lands instead of
        # waiting for all four
        # groups of <=3 live tiles (6 banks) + the v-projection tiles share
        # ONE pool: a pool close waits for all consumers (the serial ACT
        # bias chain), which previously stalled the v matmuls ~6us.
        # Bias-outs alternate ACT/Pool so neither engine's serial chain
        # paces PSUM recycling.
        all_chunks = [(ch, qb) for ch in range(4) for qb in range(NQ2)]
        with tc.tile_pool(name="ps_qkv", bufs=1, space="PSUM") as ps_qkp:

            def qk_group(chunks):
                tiles = [ps_qkp.tile([128, FBS], f32, name="psqk",
                                     tag="psqk", bufs=3) for _ in chunks]
                for c in range(4):
                    for ps, (ch, qb) in zip(tiles, chunks):
                        for ih in range(NIH):
                            hqs = slice(qb * FBS + ih * FB,
                                        qb * FBS + (ih + 1) * FB)
                            nc.tensor.matmul(
                                ps[:, ih * FB:(ih + 1) * FB],
                                wqk_sb[c][:, ch * 128:(ch + 1) * 128],
                                xt_sb[c][:, hqs],
                                start=(c == 0), stop=(c == 3))
                for ps, (ch, qb) in zip(tiles, chunks):
                    qk_bias_out(ps, ch, qb, "act")

            def v_blocks(blks):
                # ---- v projection (needed before the first head's attnV)
                for blk in blks:
                    v_block(blk)

            def v_block(blk):
                bs = slice(blk * 128, (blk + 1) * 128)
                ps = ps_qkp.tile([128, 256], f32, name="psv", tag="psv",
                                 bufs=2)
                for c in range(4):
                    nc.tensor.matmul(ps[:], xt_sb[c][:, bs], wv_sb[c][:],
                                     start=(c == 0), stop=(c == 3))
                if AV8:
                    base = (blk % 2) * HPC * VW
                    vt = v_sb[blk // 2][:, base:base + HPC * VW]
                else:
                    vt = v_sb[blk][:]
                v3 = vt.rearrange("p (h f) -> p h f", h=HPC)
                nc.vector.tensor_copy(v3[:, :, 0:64],
                                      ps.rearrange("p (h f) -> p h f", h=HPC))
                # memset lacks float32r support; write the ones/pad columns
                # through an f32 view (identical bit pattern)
                ones_ap, pad_ap = v3[:, :, 64:65], v3[:, :, 65:VW]
                if dt_attn == dt.float32r:
                    ones_ap = ones_ap.bitcast(f32)
                    pad_ap = pad_ap.bitcast(f32)
                nc.vector.memset(ones_ap, 1.0)
                nc.vector.memset(pad_ap, 0.0)

            # v interleaved between the last qkv groups so the PE has work
            # during the final xt-block DMA wait and the bias-out drains
            qk_group(all_chunks[0:3])
            qk_group(all_chunks[3:6])
            v_blocks(range(0, 8))
            qk_group(all_chunks[6:8])
            v_blocks(range(8, NKB))

        # ---- attention: one flat software-pipelined (q2, h, nk) stream.
        # attnV lags scores/exp by one step so the pipeline never drains at
        # head boundaries; head drains, denominator normalizes, and the
        # stripe projections are injected as fill steps inside later head
        # windows to keep the PE stream dense.
        # PSUM budget (8 banks): s_ps 2x2 + o_ps 1x2 + pj/bc 2x1.
        with tc.tile_pool(name="ps_s", bufs=2, space="PSUM") as ps_sp, \
                tc.tile_pool(name="ps_o", bufs=1, space="PSUM") as ps_op, \
                tc.tile_pool(name="ps_pj", bufs=2, space="PSUM") as ps_pjp, \
                tc.tile_pool(name="esbp", bufs=4) as esbp, \
                tc.tile_pool(name="stg", bufs=2) as stgp, \
                tc.tile_pool(name="osbp", bufs=3) as osbp:
            o_tiles, stgd_t = {}, {}

            def attn_v(q2, h, nk, e_sb):
                # lazy o_ps alloc: first write lands after the previous
                # head's last attnV is already emitted (bufs=1 slot reuse)
                if nk == 0:
                    o_tiles[(q2, h)] = ps_op.tile([VW if AV8 else 66, FBS],
                                                  f32, name=f"ops{h}_{q2}",
                                                  tag="ops")
                o_ps = o_tiles[(q2, h)]
                if AV8:
                    # DoubleRow: nk indexes an e/v PAIR (K = 2x128 per call)
                    lhsT = v_sb[nk].rearrange("p (j f) -> p j f",
                                              j=2)[:, :, h * VW:(h + 1) * VW]
                    e3 = e_sb.rearrange("p (j f) -> p j f", j=2)
                    for ih in range(NIH):
                        nc.tensor.matmul(
                            o_ps[:, ih * FB:(ih + 1) * FB],
                            lhsT, e3[:, :, ih * FB:(ih + 1) * FB],
                            start=(nk == 0), stop=(nk == NKB // 2 - 1),
                            perf_mode=mybir.MatmulPerfMode.DoubleRow)
                    return
                vcols = slice(h * VW, (h + 1) * VW)
                for ih in range(NIH):
                    nc.tensor.matmul(
                        o_ps[:, ih * FB:(ih + 1) * FB],
                        v_sb[nk][:, vcols],
                        e_sb[:, ih * FB:(ih + 1) * FB],
                        start=(nk == 0), stop=(nk == NKB - 1))

            def drain_head(q2, h):
                """aT copies + reciprocal of the denominator row."""
                t, odd = h // 2, h % 2
                cs = slice(q2 * FBS, (q2 + 1) * FBS)
                o_ps = o_tiles[(q2, h)]
                # lane-locked engines cannot shift partitions and DMA
                # cannot read PSUM, so shifting copies stage through SBUF
                stgd = stgp.tile([65, FBS], f32, name=f"sd{h}_{q2}",
                                 tag="stgd")
                sdb = stgp.tile([65, FBS], dt.bfloat16, name=f"sb{h}_{q2}",
                                tag="sdb")
                # den row staged first so its DMA round-trip flies while
                # the DVE does the (long) aT copies
                nc.vector.tensor_copy(stgd[64:65, :], o_ps[64:65, :])
                # den row -> denR [16, 64] block (rows h*32, col q2*64):
                # reciprocal, DMA-reshape back to a row, cast to bf16
                dblk = denR[h * 32:h * 32 + 16, q2 * 64:(q2 + 1) * 64]
                nc.sync.dma_start(dblk, stgd[64:65, :])
                if odd:
                    stg = stgp.tile([64, FBS], dt_proj, name=f"sg{h}_{q2}",
                                    tag="stg")
                    nc.vector.tensor_copy(stg[:], o_ps[0:64, :])
                    nc.sync.dma_start(aT[t][64:128, cs], stg[:])
                else:
                    nc.vector.tensor_copy(aT[t][0:64, cs], o_ps[0:64, :])
                nc.vector.reciprocal(dblk, dblk)
                sdf = stgp.tile([65, FBS], f32, name=f"sf{h}_{q2}",
                                tag="sdf")
                nc.sync.dma_start(sdf[64:65, :], dblk)
                nc.vector.tensor_copy(sdb[64:65, :], sdf[64:65, :])
                stgd_t[(q2, h)] = sdb

            def norm_half(q2, t, half):
                """PE-broadcast the 1/den rows of heads 2t,2t+1 and scale."""
                cols = slice(q2 * FBS + half * FB, q2 * FBS + (half + 1) * FB)
                bc = ps_pjp.tile([128, FB], f32, name="bc", tag="pj")
                for par in range(2):
                    sd = stgd_t[(q2, 2 * t + par)]
                    nc.tensor.matmul(
                        bc[:],
                        bcpat[64:65, par * 128:(par + 1) * 128],
                        sd[64:65, half * FB:(half + 1) * FB],
                        start=(par == 0), stop=(par == 1))
                nc.vector.tensor_mul(aT[t][:, cols], aT[t][:, cols], bc[:])
                nc.vector.tensor_scalar_add(aT[t][:, cols], aT[t][:, cols],
                                            vbv_sb[:, t:t + 1])

            def proj_blk(blk):
                bs = slice(blk * 128, (blk + 1) * 128)
                pj = ps_pjp.tile([128, FB], f32, name="pj", tag="pj")
                for f in range(2):
                    nc.tensor.matmul(pj[:], aT[f][:, bs], pjt_sb[f][:],
                                     start=(f == 0), stop=(f == 1))
                ob = osbp.tile([128, FB], f32, name="ob", tag="ob")
                nc.vector.tensor_copy(ob[:], pj[:])
                if blk >= N // 128 - 2:
                    # tail blocks: split across both queues to cut the drain
                    nc.gpsimd.dma_start(io["out"][blk * 128:blk * 128 + 64, :],
                                        ob[0:64, :])
                    nc.sync.dma_start(io["out"][blk * 128 + 64:
                                                (blk + 1) * 128, :],
                                      ob[64:128, :])
                else:
                    eng = nc.gpsimd if blk % 2 == 0 else nc.sync
                    eng.dma_start(io["out"][bs, :], ob[:])

            # fill work injected at (q2, h, nk) steps: normalizes once both
            # heads of a t-group drained (nk>=4: the reciprocal chain takes
            # ~4us after the drain pops at nk=0), stripe-0 projection spread
            # one block per two steps across stripe 1's first two windows
            fills = {
                (0, 2, 4): [lambda: norm_half(0, 0, 0)],
                (0, 2, 5): [lambda: norm_half(0, 0, 1)],
                (1, 0, 4): [lambda: norm_half(0, 1, 0)],
                (1, 0, 5): [lambda: norm_half(0, 1, 1)],
                (1, 2, 4): [lambda: norm_half(1, 0, 0)],
                (1, 2, 5): [lambda: norm_half(1, 0, 1)],
            }
            proj_steps = [(1, 0, 7), (1, 0, 9), (1, 0, 11), (1, 0, 13),
                          (1, 0, 15), (1, 1, 1), (1, 1, 3), (1, 1, 5)]
            for i, step in enumerate(proj_steps):
                fills.setdefault(step, []).append(lambda blk=i: proj_blk(blk))

            # attnV lags scores/exp via a queue: lag 3 across a head start
            # (so the previous head's o_ps drain — bufs=1 — never stalls the
            # PE), catching back up to lag 1 within the head
            pend_q = []

            NLAST = NKB // 2 - 1 if AV8 else NKB - 1

            def pop_pend():
                q2p, hp, nkp, e = pend_q.pop(0)
                attn_v(q2p, hp, nkp, e)
                if nkp == NLAST:
                    drain_head(q2p, hp)

            e_cur = None
            for q2 in range(NQ2):
                qbase = q2 * FBS
                for h in range(HPC):
                    for nk in range(NKB):
                        for f in fills.get((q2, h, nk), ()):
                            f()
                        ks = slice(nk * 128, (nk + 1) * 128)
                        s_ps = ps_sp.tile([128, FBS], f32, name="sps",
                                          tag="sps")
                        for ih in range(NIH):
                            hqs = slice(qbase + ih * FB, qbase + (ih + 1) * FB)
                            nc.tensor.matmul(s_ps[:, ih * FB:(ih + 1) * FB],
                                             KP[h][:, ks], QP[h][:, hqs],
                                             start=True, stop=True)
                        if AV8:
                            if nk % 2 == 0:
                                e_cur = esbp.tile([128, 2 * FBS], dt_av,
                                                  name="esb", tag="esb")
                            nc.scalar.activation(
                                e_cur[:, (nk % 2) * FBS:(nk % 2 + 1) * FBS],
                                s_ps[:], AF.Exp)
                            if nk % 2 == 1:
                                pend_q.append((q2, h, nk // 2, e_cur))
                                while len(pend_q) > (2 if pend_q[0][2] < 1
                                                     else 1):
                                    pop_pend()
                        else:
                            e_sb = esbp.tile([128, FBS], dt_attn, name="esb",
                                             tag="esb")
                            nc.scalar.activation(e_sb[:], s_ps[:], AF.Exp)
                            pend_q.append((q2, h, nk, e_sb))
                            while len(pend_q) > (3 if pend_q[0][2] <= 1
                                                 else 1):
                                pop_pend()
            while pend_q:
                pop_pend()
            norm_half(1, 1, 0)
            norm_half(1, 1, 1)
            for blk in range(FBS // 128, N // 128):
                proj_blk(blk)


def _build(cfg_key):
    from concourse import bacc, mybir, tile

    cfg = dict(cfg_key)
    dt = mybir.dt
    nc = bacc.Bacc("TRN2", target_bir_lowering=False, debug=False,
                   num_devices=8)
    _d = {"f32": dt.float32, "f32r": dt.float32r, "bf16": dt.bfloat16}
    dt_qkv = _d[cfg["qkv"]]
    dt_proj = _d[cfg["proj"]]
    dt_conv = dt.bfloat16 if cfg["attn"] == "bf16" else dt.float32
    shapes = {
        "xt": ([128, 4 * N], dt_qkv), "mt": ([128, 2 * N], dt_conv),
        "st": ([128, 2 * N], dt_conv),
        "wqk": ([128, 4 * 512], dt_qkv), "wv": ([128, 4 * 256], dt_qkv),
        "pjt": ([128, 2 * DIM], dt_proj),
        "mcw": ([128, 8], dt.float32),
        "scw": ([128, 8], dt.float32),
        "qkb": ([128, 4], dt.float32), "vbv": ([128, 2], dt.float32),
    }
    io = {}
    for name, (shape, dtt) in shapes.items():
        io[name] = nc.dram_tensor(name, shape, dtt,
                                  kind="ExternalInput").ap()
    io["out"] = nc.dram_tensor("out", [N, DIM], dt.float32,
                               kind="ExternalOutput").ap()
    with tile.TileContext(nc) as tc:
        _emit(tc, nc, io, cfg)
    nc.compile()
    return nc


def _get_program(cfg):
    key = tuple(sorted(cfg.items()))
    if key not in _CACHE:
        _CACHE[key] = _build(key)
    return _CACHE[key]


# ------------------------------------------------------------------ wrapper
def kernel(_cfg=None, _want_results=False, **inputs):
    from concourse.bass_utils import run_bass_kernel_spmd

    cfg = dict(_DEFAULT_CFG)
    if _cfg:
        cfg.update(_cfg)
    env_cfg = os.environ.get("BASSKERN_CFG")
    if env_cfg:  # e.g. "attn=f32r,qkv=f32r"
        for kv in env_cfg.split(","):
            k, v = kv.split("=")
            cfg[k] = v

    inputs = {k: np.asarray(v, dtype=np.float32) for k, v in inputs.items()}
    nc = _get_program(cfg)
    in_maps = [_host_prep(core, inputs, cfg) for core in range(8)]
    res = run_bass_kernel_spmd(nc, in_maps, list(range(8)))

    out = np.empty((B, N, DIM), np.float32)
    pb = inputs["proj_b"]
    for b in range(B):
        out[b] = res.results[2 * b]["out"] + res.results[2 * b + 1]["out"] + pb
    if _want_results:
        return out, res
    return out


# revision 66
# speedup vs baseline: 1.0106x; 1.0106x over previous
"""Trainium2 Bass kernel for nn_AttentionWithVQ (B=4, N=2048, DIM=512, H=8,
depthwise-conv positional term, softmax attention, output projection).

Sharding: data-parallel over B (4 batches x 2 core-groups) and tensor-parallel
over heads (4 heads per core) -> 8 cores, fully independent per core except a
final partial-sum over the two head-groups of each batch, done on host at
gather time (the output projection contracts over heads).

Core algorithmic fusion: the score matrix
    S = 0.5*(scale * q @ k^T + scale * conv1(m) @ conv2(s)^T)
is ONE matmul over a concatenated 128-feature axis:
    S = Qp^T @ Kp,  Qp = [q*scale*0.5 ; conv1(m)*scale*0.5], Kp = [k ; conv2(s)]
which exactly fills the 128x128 PE array contraction dim.

Softmax denominators come for free by appending a ones-column to V
(attn@[V|1] yields the row-sums of exp(S) in the last output row); exp() is
numerically safe without max-subtraction for this problem's score magnitudes
(|S| < ~1 given the 0.02-scaled weights).

Partition alignment: compute engines are lane-locked (PSUM partition p ->
SBUF partition p), so per-head feature layouts alternate by head parity
(even heads [qk;conv], odd heads [conv;qk]) making every PSUM->SBUF copy
partition-aligned; the few genuinely shifting copies (odd-head attention
outputs, denominator rows) go through DMA, which can move partitions freely.
All permutation bookkeeping is done host-side in numpy when preparing
per-core inputs.

Schedule (v2): the kernel is a single software-pipelined stream ordered to
keep the PE and ACT engines saturated end-to-end:
  - input DMAs are split across the SP/ACT/Pool queues with the
    qkv-critical tensors (xt, wqk) first;
  - qkv bias-adds run on the ACT engine (idle until the first exp);
  - attention runs stripe-outer (q 1024-blocks) / head-inner, with
    scores(nk+1) emitted before attnV(nk) so exp latency is hidden, the
    v-projection matmuls interleaved into the first head's window and the
    second half of the q/k projection into the second head's window;
  - each stripe is normalized, projected, and DMA'd out as soon as its
    4 heads finish, overlapping the next stripe's attention.
"""

import os
import sys

sys.path.insert(0, "/opt/trn_rl_repo")

import numpy as np

# ---------------------------------------------------------------- constants
B, N, DIM, HEAD, VQE_K = 4, 2048, 512, 8, 3
Dh = DIM // HEAD            # 64
HPC = HEAD // 2             # heads per core (8 cores = 4 batch * 2 groups)
P = 128
FB = 512                    # one fp32 PSUM bank
NKB = N // P                # 16
SCALE_Q = Dh ** -0.5 * 0.5  # folds the 0.5 score scale into the q/conv1 side

_DEFAULT_CFG = {"qkv": "bf16", "attn": "bf16", "proj": "bf16", "av": "same"}

_CACHE = {}


def _np_dt(v):
    if v == "bf16":
        import ml_dtypes
        return ml_dtypes.bfloat16
    return np.float32


# ---------------------------------------------------------------- host prep
def _host_prep(core, inp, cfg=None):
    """Build the per-core input arrays (sharding + layout permutations)."""
    cfg = cfg or _DEFAULT_CFG
    b, g = core // 2, core % 2
    f32 = np.float32
    x, m, s = inp["x"], inp["m"], inp["s"]
    qkv_w, qkv_b = inp["qkv_w"], inp["qkv_b"]
    proj_w = inp["proj_w"]
    p1w = inp["pe1_w"].reshape(HEAD, VQE_K)
    p2w = inp["pe2_w"].reshape(HEAD, VQE_K)
    pe1_b, pe2_b = inp["pe1_b"], inp["pe2_b"]
    dt_qkv = _np_dt(cfg["qkv"])
    dt_proj = _np_dt(cfg["proj"])
    dt_conv = _np_dt("bf16" if cfg["attn"] == "bf16" else "f32")

    def fold(a):
        """[k*128, C] -> [128, k*C]: partition p holds rows p, 128+p, ...
        concatenated, so each partition's DMA data is one contiguous run."""
        k = a.shape[0] // 128
        return np.ascontiguousarray(
            np.concatenate([a[c * 128:(c + 1) * 128] for c in range(k)],
                           axis=1))

    d = {}
    d["xt"] = fold(x[b].T.astype(dt_qkv))  # [128, 4*2048]

    # m/s transposed, tile t rows = [head(2t+1) feats ; head(2t) feats]
    mt = np.empty((256, N), f32)
    st = np.empty((256, N), f32)
    mcw = np.zeros((128, 8), f32)
    scw = np.zeros((128, 8), f32)
    for t in range(2):
        h_lo, h_hi = g * 4 + 2 * t + 1, g * 4 + 2 * t
        mt[t * 128:t * 128 + 64] = m[b][:, h_lo * 64:(h_lo + 1) * 64].T
        mt[t * 128 + 64:t * 128 + 128] = m[b][:, h_hi * 64:(h_hi + 1) * 64].T
        st[t * 128:t * 128 + 64] = s[b][:, h_lo * 64:(h_lo + 1) * 64].T
        st[t * 128 + 64:t * 128 + 128] = s[b][:, h_hi * 64:(h_hi + 1) * 64].T
        for p in range(128):
            h = g * 4 + 2 * t + (1 if p < 64 else 0)
            mcw[p, 4 * t:4 * t + 3] = p1w[h] * SCALE_Q
            scw[p, 4 * t:4 * t + 3] = p2w[h]
            mcw[p, 4 * t + 3] = pe1_b[h] * SCALE_Q
            scw[p, 4 * t + 3] = pe2_b[h]
    d["mt"], d["st"] = fold(mt.astype(dt_conv)), fold(st.astype(dt_conv))
    d["mcw"], d["scw"] = mcw, scw

    # q/k projection weights: chunk ch=(t, q|k) = [even-head rows; odd-head rows]
    wqk_f = np.empty((512, DIM), f32)
    qkb = np.zeros((128, 4), f32)
    for t in range(2):
        for j in range(2):  # 0=q, 1=k
            ch = 2 * t + j
            h_e, h_o = g * 4 + 2 * t, g * 4 + 2 * t + 1
            base = j * DIM
            wqk_f[ch * 128:ch * 128 + 64] = qkv_w[base + h_e * 64:base + (h_e + 1) * 64]
            wqk_f[ch * 128 + 64:(ch + 1) * 128] = qkv_w[base + h_o * 64:base + (h_o + 1) * 64]
            qkb[0:64, ch] = qkv_b[base + h_e * 64:base + (h_e + 1) * 64]
            qkb[64:128, ch] = qkv_b[base + h_o * 64:base + (h_o + 1) * 64]
            if j == 0:
                wqk_f[ch * 128:(ch + 1) * 128] *= SCALE_Q
                qkb[:, ch] *= SCALE_Q
    d["wqk"] = fold(wqk_f.T.astype(dt_qkv))  # [128, 4*512]
    d["qkb"] = qkb

    d["wv"] = fold(np.ascontiguousarray(
        qkv_w[2 * DIM + g * 256:2 * DIM + (g + 1) * 256].T).astype(dt_qkv))

    # proj rows / v-bias in aT partition order:
    # aT tile t partition p -> head 2t+(p>=64), d=p%64
    pjt = np.empty((256, DIM), f32)
    vbv = np.empty((256,), f32)
    for t in range(2):
        for p in range(128):
            h_l = 2 * t + (1 if p >= 64 else 0)
            h = g * 4 + h_l
            pjt[t * 128 + p] = proj_w[:, h * 64 + (p % 64)]
            vbv[t * 128 + p] = qkv_b[2 * DIM + h * 64 + (p % 64)]
    d["pjt"] = fold(pjt.astype(dt_proj))  # [128, 2*512]
    d["vbv"] = np.ascontiguousarray(vbv.reshape(2, 128).T.astype(f32))  # [128, 2]
    return d


# ------------------------------------------------------------- device build
def _emit(tc, nc, io, cfg):
    from contextlib import ExitStack

    from concourse import mybir

    dt = mybir.dt
    f32 = dt.float32
    AF = mybir.ActivationFunctionType
    ALU = mybir.AluOpType

    def _dt(v):
        return {"f32": f32, "f32r": dt.float32r, "bf16": dt.bfloat16}[v]

    dt_qkv = _dt(cfg["qkv"])
    dt_attn = _dt(cfg["attn"])
    dt_proj = _dt(cfg["proj"])
    dt_conv = dt.bfloat16 if dt_attn == dt.bfloat16 else f32
    # av=f8: exp writes fp8e4m3 and attnV runs DoubleRow (K=256/instr,
    # 2x PE throughput); scores/QP/KP stay in dt_attn
    AV8 = cfg.get("av") == "f8"
    dt_av = dt.float8e4 if AV8 else dt_attn
    VW = 68 if AV8 else 66  # per-head V block width (4B-aligned offsets)
    # exp granularity: one ACT op per FBS-wide stripe (matmuls within are
    # still 512-wide: a matmul output cannot cross a PSUM bank)
    FBS = 1024
    NIH = FBS // FB           # 2
    NQ2 = N // FBS            # 2 q-stripes

    with ExitStack() as ctx:
        persist = ctx.enter_context(tc.tile_pool(name="persist", bufs=1))

        # ---- persistent tiles
        QP, KP, v_sb, aT = [], [], [], []
        # critical-path inputs first: xt on the SP queue, wqk on the ACT
        # queue so both stream concurrently from t=0
        # inputs are host-folded to [128, k*cols] (each partition's bytes
        # one contiguous DRAM run -> ~4x fewer DMA descriptors); weight
        # tiles first on each HWDGE queue, xt halves split across both
        xt_big = persist.tile([128, 4 * N], dt_qkv, name="xt", tag="xt")
        xt_sb = [xt_big[:, c * N:(c + 1) * N] for c in range(4)]
        wqk_big = persist.tile([128, 4 * 512], dt_qkv, name="wqk", tag="wqk")
        wqk_sb = [wqk_big[:, c * 512:(c + 1) * 512] for c in range(4)]
        wv_big = persist.tile([128, 4 * 256], dt_qkv, name="wv", tag="wv")
        wv_sb = [wv_big[:, c * 256:(c + 1) * 256] for c in range(4)]
        # pieces sized/ordered so xt block arrivals are evenly spaced in
        # c-loop consumption order across both HWDGE queues
        nc.scalar.dma_start(wqk_big[:, 0:1024], io["wqk"][:, 0:1024])
        nc.sync.dma_start(xt_big[:, 0:N], io["xt"][:, 0:N])
        nc.scalar.dma_start(xt_big[:, N:2 * N], io["xt"][:, N:2 * N])
        nc.sync.dma_start(xt_big[:, 2 * N:3 * N], io["xt"][:, 2 * N:3 * N])
        nc.scalar.dma_start(wqk_big[:, 1024:2048], io["wqk"][:, 1024:2048])
        nc.gpsimd.dma_start(wv_big[:], io["wv"][:, :])
        nc.scalar.dma_start(xt_big[:, 3 * N:4 * N], io["xt"][:, 3 * N:4 * N])
        mcw_sb = persist.tile([128, 8], f32, name="mcw", tag="mcw")
        nc.gpsimd.dma_start(mcw_sb[:], io["mcw"][:, :])
        scw_sb = persist.tile([128, 8], f32, name="scw", tag="scw")
        nc.gpsimd.dma_start(scw_sb[:], io["scw"][:, :])
        qkb_sb = persist.tile([128, 4], f32, name="qkb", tag="qkb")
        nc.sync.dma_start(qkb_sb[:], io["qkb"][:, :])

        for h in range(HPC):
            QP.append(persist.tile([128, N], dt_attn, name=f"QP{h}", tag=f"QP{h}"))
            KP.append(persist.tile([128, N], dt_attn, name=f"KP{h}", tag=f"KP{h}"))
        # per-head V block is [v(64) | ones | zero-pad] = VW columns: matmul
        # operands need 4-byte-aligned offsets, so the block width must keep
        # h*VW*dtsize 4-aligned (66 for 2-byte dtypes, 68 for fp8)
        if AV8:
            for p in range(NKB // 2):
                v_sb.append(persist.tile([128, 2 * HPC * VW], dt_av,
                                         name=f"vsb{p}", tag=f"vsb{p}"))
        else:
            for blk in range(NKB):
                v_sb.append(persist.tile([128, HPC * VW], dt_attn,
                                         name=f"vsb{blk}", tag=f"vsb{blk}"))
        for t in range(2):
            aT.append(persist.tile([128, N], dt_proj, name=f"aT{t}", tag=f"aT{t}"))
        # softmax denominators, DMA-reshaped onto 16-partition tiles so the
        # (slow per-element) reciprocal runs 16x wider than a row layout;
        # head h at rows h*32 (32-aligned for the DVE), stripe in columns
        denR = persist.tile([128, 128], f32, name="denR", tag="denR")
        # stationary 0/1 patterns for the denominator-broadcast matmul:
        # lhsT = bcpat[64:65, par*128:(par+1)*128] replicates a [1, FB] den
        # row onto the head's 64-partition half of a PSUM tile (bf16: the
        # BIR verifier requires f32r operands to be produced as f32r, which
        # memset/copy cannot do)
        bcpat = persist.tile([128, 256], dt.bfloat16, name="bcpat",
                             tag="bcpat")
        nc.vector.memset(bcpat[:, 0:64], 1.0)
        nc.vector.memset(bcpat[:, 64:192], 0.0)
        nc.vector.memset(bcpat[:, 192:256], 1.0)

        vbv_sb = persist.tile([128, 2], f32, name="vbv", tag="vbv")
        nc.sync.dma_start(vbv_sb[:], io["vbv"][:, :])
        pjt_big = persist.tile([128, 2 * 512], dt_proj, name="pjt", tag="pjt")
        pjt_sb = [pjt_big[:, f * 512:(f + 1) * 512] for f in range(2)]
        nc.scalar.dma_start(pjt_big[:], io["pjt"][:, :])

        # ---- depthwise convs (DVE, bf16 2x mode; inputs on the Pool queue)
        # PE p-state warm-up: ~10 back-to-back dummy matmuls on the (tiny,
        # already-memset) bcpat tile during the input-DMA wait, so the real
        # qkv matmuls start at full clock instead of ramping through them
        with tc.tile_pool(name="ps_warm", bufs=1, space="PSUM") as ps_wp:
            wps = ps_wp.tile([128, 256], f32, name="warm", tag="warm")
            # sized to keep the PE clocked up until the first xt piece lands
            for _ in range(44):
                nc.tensor.matmul(wps[:], bcpat[:, 0:128], bcpat[:, 0:256],
                                 start=True, stop=True)


        # t=0 convs first: heads 0/1 gate the first attention window; the
        # folded [128, 2N] inputs are loaded whole and convolved in place
        mt_sb = persist.tile([128, 2 * N], dt_conv, name="mt", tag="mt")
        nc.gpsimd.dma_start(mt_sb[:], io["mt"][:, :])
        st_sb = persist.tile([128, 2 * N], dt_conv, name="st", tag="st")
        nc.gpsimd.dma_start(st_sb[:], io["st"][:, :])
        with tc.tile_pool(name="conv", bufs=2) as convp:
            for t in range(2):
                for src_sb, wv_, dst in ((mt_sb, mcw_sb, QP),
                                         (st_sb, scw_sb, KP)):
                    xin = src_sb[:, t * N:(t + 1) * N]
                    y = convp.tile([128, N], dt_conv, name=f"cy_{t}",
                                   tag="cy", bufs=2)
                    w0, w1, w2, cb = (wv_[:, 4 * t + k:4 * t + k + 1]
                                      for k in range(4))
                    nc.vector.tensor_scalar(y[:], xin[:], w1, cb,
                                            ALU.mult, ALU.add)
                    nc.vector.scalar_tensor_tensor(
                        y[:, 1:], xin[:, :N - 1], w0, y[:, 1:],
                        ALU.mult, ALU.add)
                    nc.vector.scalar_tensor_tensor(
                        y[:, :N - 1], xin[:, 1:], w2, y[:, :N - 1],
                        ALU.mult, ALU.add)
                    nc.vector.tensor_copy(dst[2 * t + 1][0:64, :], y[0:64, :])
                    nc.vector.tensor_copy(dst[2 * t][64:128, :], y[64:128, :])

        # ---- q/k projections, first half (chunks 0,1 = heads 0,1).
        # Bias-adds run on ACT (idle until the first exp); the PSUM pool
        # closes before attention so its banks are reusable there.
        def qk_chunk_matmuls(ps, ch, qb):
            for ih in range(NIH):
                hqs = slice(qb * FBS + ih * FB, qb * FBS + (ih + 1) * FB)
                for c in range(4):
                    nc.tensor.matmul(
                        ps[:, ih * FB:(ih + 1) * FB],
                        wqk_sb[c][:, ch * 128:(ch + 1) * 128],
                        xt_sb[c][:, hqs],
                        start=(c == 0), stop=(c == 3))

        def qk_bias_out(ps, ch, qb, engine):
            t = ch // 2
            qs = slice(qb * FBS, (qb + 1) * FBS)
            dst = QP if ch % 2 == 0 else KP
            if engine == "act":
                nc.scalar.activation(dst[2 * t][0:64, qs], ps[0:64, :],
                                     AF.Identity, bias=qkb_sb[0:64, ch:ch + 1])
                nc.scalar.activation(dst[2 * t + 1][64:128, qs], ps[64:128, :],
                                     AF.Identity,
                                     bias=qkb_sb[64:128, ch:ch + 1])
            elif engine == "pool":
                nc.gpsimd.tensor_scalar_add(dst[2 * t][0:64, qs],
                                            ps[0:64, :],
                                            qkb_sb[0:64, ch:ch + 1])
                nc.gpsimd.tensor_scalar_add(dst[2 * t + 1][64:128, qs],
                                            ps[64:128, :],
                                            qkb_sb[64:128, ch:ch + 1])
            else:
                nc.vector.tensor_scalar_add(dst[2 * t][0:64, qs], ps[0:64, :],
                                            qkb_sb[0:64, ch:ch + 1])
                nc.vector.tensor_scalar_add(dst[2 * t + 1][64:128, qs],
                                            ps[64:128, :],
                                            qkb_sb[64:128, ch:ch + 1])

        # c-OUTER accumulation over 4 live PSUM tiles (8 banks): matmuls for
        # xt block c start as soon as that block's DMA lands instead of
        # waiting for all four
        # groups of <=3 live tiles (6 banks) + the v-projection tiles share
        # ONE pool: a pool close waits for all consumers (the serial ACT
        # bias chain), which previously stalled the v matmuls ~6us.
        # Bias-outs alternate ACT/Pool so neither engine's serial chain
        # paces PSUM recycling.
        all_chunks = [(ch, qb) for ch in range(4) for qb in range(NQ2)]
        with tc.tile_pool(name="ps_qkv", bufs=1, space="PSUM") as ps_qkp:

            def qk_group(chunks):
                tiles = [ps_qkp.tile([128, FBS], f32, name="psqk",
                                     tag="psqk", bufs=3) for _ in chunks]
                for c in range(4):
                    for ps, (ch, qb) in zip(tiles, chunks):
                        for ih in range(NIH):
                            hqs = slice(qb * FBS + ih * FB,
                                        qb * FBS + (ih + 1) * FB)
                            nc.tensor.matmul(
                                ps[:, ih * FB:(ih + 1) * FB],
                                wqk_sb[c][:, ch * 128:(ch + 1) * 128],
                                xt_sb[c][:, hqs],
                                start=(c == 0), stop=(c == 3))
                for ps, (ch, qb) in zip(tiles, chunks):
                    qk_bias_out(ps, ch, qb, "act")

            def v_blocks(blks):
                # ---- v projection (needed before the first head's attnV)
                for blk in blks:
                    v_block(blk)

            def v_block(blk):
                bs = slice(blk * 128, (blk + 1) * 128)
                ps = ps_qkp.tile([128, 256], f32, name="psv", tag="psv",
                                 bufs=2)
                for c in range(4):
                    nc.tensor.matmul(ps[:], xt_sb[c][:, bs], wv_sb[c][:],
                                     start=(c == 0), stop=(c == 3))
                if AV8:
                    base = (blk % 2) * HPC * VW
                    vt = v_sb[blk // 2][:, base:base + HPC * VW]
                else:
                    vt = v_sb[blk][:]
                v3 = vt.rearrange("p (h f) -> p h f", h=HPC)
                nc.vector.tensor_copy(v3[:, :, 0:64],
                                      ps.rearrange("p (h f) -> p h f", h=HPC))
                # memset lacks float32r support; write the ones/pad columns
                # through an f32 view (identical bit pattern)
                ones_ap, pad_ap = v3[:, :, 64:65], v3[:, :, 65:VW]
                if dt_attn == dt.float32r:
                    ones_ap = ones_ap.bitcast(f32)
                    pad_ap = pad_ap.bitcast(f32)
                nc.vector.memset(ones_ap, 1.0)
                nc.vector.memset(pad_ap, 0.0)

            # v interleaved between the last qkv groups so the PE has work
            # during the final xt-block DMA wait and the bias-out drains
            qk_group(all_chunks[0:3])
            qk_group(all_chunks[3:6])
            v_blocks(range(0, 8))
            qk_group(all_chunks[6:8])
            v_blocks(range(8, NKB))

        # ---- attention: one flat software-pipelined (q2, h, nk) stream.
        # attnV lags scores/exp by one step so the pipeline never drains at
        # head boundaries; head drains, denominator normalizes, and the
        # stripe projections are injected as fill steps inside later head
        # windows to keep the PE stream dense.
        # PSUM budget (8 banks): s_ps 2x2 + o_ps 1x2 + pj/bc 2x1.
        with tc.tile_pool(name="ps_s", bufs=2, space="PSUM") as ps_sp, \
                tc.tile_pool(name="ps_o", bufs=1, space="PSUM") as ps_op, \
                tc.tile_pool(name="ps_pj", bufs=2, space="PSUM") as ps_pjp, \
                tc.tile_pool(name="esbp", bufs=4) as esbp, \
                tc.tile_pool(name="stg", bufs=2) as stgp, \
                tc.tile_pool(name="osbp", bufs=3) as osbp:
            o_tiles, stgd_t = {}, {}

            def attn_v(q2, h, nk, e_sb):
                # lazy o_ps alloc: first write lands after the previous
                # head's last attnV is already emitted (bufs=1 slot reuse)
                if nk == 0:
                    o_tiles[(q2, h)] = ps_op.tile([VW if AV8 else 66, FBS],
                                                  f32, name=f"ops{h}_{q2}",
                                                  tag="ops")
                o_ps = o_tiles[(q2, h)]
                if AV8:
                    # DoubleRow: nk indexes an e/v PAIR (K = 2x128 per call)
                    lhsT = v_sb[nk].rearrange("p (j f) -> p j f",
                                              j=2)[:, :, h * VW:(h + 1) * VW]
                    e3 = e_sb.rearrange("p (j f) -> p j f", j=2)
                    for ih in range(NIH):
                        nc.tensor.matmul(
                            o_ps[:, ih * FB:(ih + 1) * FB],
                            lhsT, e3[:, :, ih * FB:(ih + 1) * FB],
                            start=(nk == 0), stop=(nk == NKB // 2 - 1),
                            perf_mode=mybir.MatmulPerfMode.DoubleRow)
                    return
                vcols = slice(h * VW, (h + 1) * VW)
                for ih in range(NIH):
                    nc.tensor.matmul(
                        o_ps[:, ih * FB:(ih + 1) * FB],
                        v_sb[nk][:, vcols],
                        e_sb[:, ih * FB:(ih + 1) * FB],
                        start=(nk == 0), stop=(nk == NKB - 1))

            def drain_head(q2, h):
                """aT copies + reciprocal of the denominator row."""
                t, odd = h // 2, h % 2
                cs = slice(q2 * FBS, (q2 + 1) * FBS)
                o_ps = o_tiles[(q2, h)]
                # lane-locked engines cannot shift partitions and DMA
                # cannot read PSUM, so shifting copies stage through SBUF
                stgd = stgp.tile([65, FBS], f32, name=f"sd{h}_{q2}",
                                 tag="stgd")
                sdb = stgp.tile([65, FBS], dt.bfloat16, name=f"sb{h}_{q2}",
                                tag="sdb")
                # den row staged first so its DMA round-trip flies while
                # the DVE does the (long) aT copies
                nc.vector.tensor_copy(stgd[64:65, :], o_ps[64:65, :])
                # den row -> denR [16, 64] block (rows h*32, col q2*64):
                # reciprocal, DMA-reshape back to a row, cast to bf16
                dblk = denR[h * 32:h * 32 + 16, q2 * 64:(q2 + 1) * 64]
                nc.sync.dma_start(dblk, stgd[64:65, :])
                if odd:
                    stg = stgp.tile([64, FBS], dt_proj, name=f"sg{h}_{q2}",
                                    tag="stg")
                    nc.vector.tensor_copy(stg[:], o_ps[0:64, :])
                    nc.sync.dma_start(aT[t][64:128, cs], stg[:])
                else:
                    nc.vector.tensor_copy(aT[t][0:64, cs], o_ps[0:64, :])
                nc.vector.reciprocal(dblk, dblk)
                sdf = stgp.tile([65, FBS], f32, name=f"sf{h}_{q2}",
                                tag="sdf")
                nc.sync.dma_start(sdf[64:65, :], dblk)
                nc.vector.tensor_copy(sdb[64:65, :], sdf[64:65, :])
                stgd_t[(q2, h)] = sdb

            def norm_half(q2, t, half):
                """PE-broadcast the 1/den rows of heads 2t,2t+1 and scale."""
                cols = slice(q2 * FBS + half * FB, q2 * FBS + (half + 1) * FB)
                bc = ps_pjp.tile([128, FB], f32, name="bc", tag="pj")
                for par in range(2):
                    sd = stgd_t[(q2, 2 * t + par)]
                    nc.tensor.matmul(
                        bc[:],
                        bcpat[64:65, par * 128:(par + 1) * 128],
                        sd[64:65, half * FB:(half + 1) * FB],
                        start=(par == 0), stop=(par == 1))
                nc.vector.tensor_mul(aT[t][:, cols], aT[t][:, cols], bc[:])
                nc.vector.tensor_scalar_add(aT[t][:, cols], aT[t][:, cols],
                                            vbv_sb[:, t:t + 1])

            def proj_blk(blk):
                bs = slice(blk * 128, (blk + 1) * 128)
                pj = ps_pjp.tile([128, FB], f32, name="pj", tag="pj")
                for f in range(2):
                    nc.tensor.matmul(pj[:], aT[f][:, bs], pjt_sb[f][:],
                                     start=(f == 0), stop=(f == 1))
                ob = osbp.tile([128, FB], f32, name="ob", tag="ob")
                nc.vector.tensor_copy(ob[:], pj[:])
                if blk >= FBS // 128:
                    # tail blocks: split across both queues to cut the drain
                    nc.gpsimd.dma_start(io["out"][blk * 128:blk * 128 + 64, :],
                                        ob[0:64, :])
                    nc.sync.dma_start(io["out"][blk * 128 + 64:
                                                (blk + 1) * 128, :],
                                      ob[64:128, :])
                else:
                    eng = nc.gpsimd if blk % 2 == 0 else nc.sync
                    eng.dma_start(io["out"][bs, :], ob[:])

            # fill work injected at (q2, h, nk) steps: normalizes once both
            # heads of a t-group drained (nk>=4: the reciprocal chain takes
            # ~4us after the drain pops at nk=0), stripe-0 projection spread
            # one block per two steps across stripe 1's first two windows
            fills = {
                (0, 2, 4): [lambda: norm_half(0, 0, 0)],
                (0, 2, 5): [lambda: norm_half(0, 0, 1)],
                (1, 0, 4): [lambda: norm_half(0, 1, 0)],
                (1, 0, 5): [lambda: norm_half(0, 1, 1)],
                (1, 2, 4): [lambda: norm_half(1, 0, 0)],
                (1, 2, 5): [lambda: norm_half(1, 0, 1)],
            }
            proj_steps = [(1, 0, 7), (1, 0, 9), (1, 0, 11), (1, 0, 13),
                          (1, 0, 15), (1, 1, 1), (1, 1, 3), (1, 1, 5)]
            for i, step in enumerate(proj_steps):
                fills.setdefault(step, []).append(lambda blk=i: proj_blk(blk))

            # attnV lags scores/exp via a queue: lag 3 across a head start
            # (so the previous head's o_ps drain — bufs=1 — never stalls the
            # PE), catching back up to lag 1 within the head
            pend_q = []

            NLAST = NKB // 2 - 1 if AV8 else NKB - 1

            def pop_pend():
                q2p, hp, nkp, e = pend_q.pop(0)
                attn_v(q2p, hp, nkp, e)
                if nkp == NLAST:
                    drain_head(q2p, hp)

            e_cur = None
            for q2 in range(NQ2):
                qbase = q2 * FBS
                for h in range(HPC):
                    for nk in range(NKB):
                        for f in fills.get((q2, h, nk), ()):
                            f()
                        ks = slice(nk * 128, (nk + 1) * 128)
                        s_ps = ps_sp.tile([128, FBS], f32, name="sps",
                                          tag="sps")
                        for ih in range(NIH):
                            hqs = slice(qbase + ih * FB, qbase + (ih + 1) * FB)
                            nc.tensor.matmul(s_ps[:, ih * FB:(ih + 1) * FB],
                                             KP[h][:, ks], QP[h][:, hqs],
                                             start=True, stop=True)
                        if AV8:
                            if nk % 2 == 0:
                                e_cur = esbp.tile([128, 2 * FBS], dt_av,
                                                  name="esb", tag="esb")
                            nc.scalar.activation(
                                e_cur[:, (nk % 2) * FBS:(nk % 2 + 1) * FBS],
                                s_ps[:], AF.Exp)
                            if nk % 2 == 1:
                                pend_q.append((q2, h, nk // 2, e_cur))
                                while len(pend_q) > (2 if pend_q[0][2] < 1
                                                     else 1):
                                    pop_pend()
                        else:
                            e_sb = esbp.tile([128, FBS], dt_attn, name="esb",
                                             tag="esb")
                            nc.scalar.activation(e_sb[:], s_ps[:], AF.Exp)
                            pend_q.append((q2, h, nk, e_sb))
                            while len(pend_q) > (3 if pend_q[0][2] <= 1
                                                 else 1):
                                pop_pend()
            while pend_q:
                pop_pend()
            norm_half(1, 1, 0)
            norm_half(1, 1, 1)
            for blk in range(FBS // 128, N // 128):
                proj_blk(blk)


def _build(cfg_key):
    from concourse import bacc, mybir, tile

    cfg = dict(cfg_key)
    dt = mybir.dt
    nc = bacc.Bacc("TRN2", target_bir_lowering=False, debug=False,
                   num_devices=8)
    _d = {"f32": dt.float32, "f32r": dt.float32r, "bf16": dt.bfloat16}
    dt_qkv = _d[cfg["qkv"]]
    dt_proj = _d[cfg["proj"]]
    dt_conv = dt.bfloat16 if cfg["attn"] == "bf16" else dt.float32
    shapes = {
        "xt": ([128, 4 * N], dt_qkv), "mt": ([128, 2 * N], dt_conv),
        "st": ([128, 2 * N], dt_conv),
        "wqk": ([128, 4 * 512], dt_qkv), "wv": ([128, 4 * 256], dt_qkv),
        "pjt": ([128, 2 * DIM], dt_proj),
        "mcw": ([128, 8], dt.float32),
        "scw": ([128, 8], dt.float32),
        "qkb": ([128, 4], dt.float32), "vbv": ([128, 2], dt.float32),
    }
    io = {}
    for name, (shape, dtt) in shapes.items():
        io[name] = nc.dram_tensor(name, shape, dtt,
                                  kind="ExternalInput").ap()
    io["out"] = nc.dram_tensor("out", [N, DIM], dt.float32,
                               kind="ExternalOutput").ap()
    with tile.TileContext(nc) as tc:
        _emit(tc, nc, io, cfg)
    nc.compile()
    return nc


def _get_program(cfg):
    key = tuple(sorted(cfg.items()))
    if key not in _CACHE:
        _CACHE[key] = _build(key)
    return _CACHE[key]


# ------------------------------------------------------------------ wrapper
def kernel(_cfg=None, _want_results=False, **inputs):
    from concourse.bass_utils import run_bass_kernel_spmd

    cfg = dict(_DEFAULT_CFG)
    if _cfg:
        cfg.update(_cfg)
    env_cfg = os.environ.get("BASSKERN_CFG")
    if env_cfg:  # e.g. "attn=f32r,qkv=f32r"
        for kv in env_cfg.split(","):
            k, v = kv.split("=")
            cfg[k] = v

    inputs = {k: np.asarray(v, dtype=np.float32) for k, v in inputs.items()}
    nc = _get_program(cfg)
    in_maps = [_host_prep(core, inputs, cfg) for core in range(8)]
    res = run_bass_kernel_spmd(nc, in_maps, list(range(8)))

    out = np.empty((B, N, DIM), np.float32)
    pb = inputs["proj_b"]
    for b in range(B):
        out[b] = res.results[2 * b]["out"] + res.results[2 * b + 1]["out"] + pb
    if _want_results:
        return out, res
    return out
